# revision 24
# baseline (speedup 1.0000x reference)
"""ComplementaryLIFNeuron on 8 Trainium2 NeuronCores (Bass, raw engine blocks).

Reference recurrence (per time step t, elementwise over [b, n, c]):
    v = v * 0.5 + x
    p = sigmoid(v / 2)          # 0.5 + 0.5*tanh(v/4)
    m = m * p
    s = (v >= 1)
    m = m + s
    q = sigmoid(m)              # 0.5 + 0.5*tanh(m/2)
    v = (v - s) - s * q
Output is s for each step, shape [(t*b), n, c].

Sharding: data-parallel over batch b=32 -> 4 rows per core; each (t, core)
block is a contiguous [4, 196*768] = [128, 4704] fp32 chunk, split into
2 interleaved column streams of 2352.

Exactness strategy (bit-identical to XLA fp32 except the tanh tables):
    m*p == 0.5 * ((tanh+1)*m)   (pow2 scaling commutes with RNE)
    track z = -((v-s) - s*q)    (negation symmetry of RNE), so
    s - v, + s*q and the next charge v' = (-0.5)*z + x all match exactly.
    t=0: v0 = x, s0 = (x>=1), m1 = s0, q0 = s0 * sigma(1) with sigma(1)
    hardcoded to XLA's fp32 bit pattern. t=3: only v3, s3 computed.

Engine split (per NeuronCore):
    DVE    : all 2-input STT/TT chain ops (never touch the shared SBUF port)
    GPSIMD : spike compare (tensor_scalar is_ge) + vt=(v>=1)-v STT
    ACT    : tanh's, sigma(1) scaling, spike store DMA issue (HWDGE)
    SYNC   : input load DMA issue (HWDGE)
"""

import sys
import types
import numpy as np

STEP = 4
B = 32
N = 196
C = 768
NCORES = 8
BPC = B // NCORES            # batch rows per core = 4
PELEM = BPC * N * C          # elements per (t, core) block = 602112
P = 128                      # SBUF partitions
FDFULL = PELEM // P          # 4704 free-dim columns per (t, core)
NSTREAM = 2                  # independent column streams (interleaved)
FD = FDFULL // NSTREAM       # 2352 columns per stream tile

SIGMA1 = float(np.uint32(0x3F3B26A8).view(np.float32))  # XLA fp32 sigmoid(1.0)

USE_PE = True         # TensorEngine computes m = 0.5*w + s (bit-exact)
CHUNK = 512           # PSUM bank = 512 fp32 columns
CHUNKS = [(c, min(CHUNK, FD - c)) for c in range(0, FD, CHUNK)]

_CACHE = {}


def _ensure_axon_hooks():
    """bass_utils' trace path imports antenv.axon_hooks, absent in this image.

    Recreate the module and register the ctypes NTFF hook that
    trn_agent_boot would have installed if the module existed.
    """
    import antenv

    if "antenv.axon_hooks" not in sys.modules:
        m = types.ModuleType("antenv.axon_hooks")
        hook = [None]
        m.set_axon_ntff_profile_hook = lambda h: hook.__setitem__(0, h)
        m.get_axon_ntff_profile_hook = lambda: hook[0]
        sys.modules["antenv.axon_hooks"] = m
        antenv.axon_hooks = m
        try:
            from trn_agent_boot.trn_boot import _ntff_profile_via_ctypes

            h = _ntff_profile_via_ctypes("/opt/axon/libaxon_pjrt.so")
            if h is not None:
                m.set_axon_ntff_profile_hook(h)
        except Exception:
            pass


def build_bass():
    """Build the per-core SPMD Bass program."""
    from concourse import bass
    import concourse.mybir as mybir

    fp32 = mybir.dt.float32
    Alu = mybir.AluOpType
    Act = mybir.ActivationFunctionType

    nc = bass.Bass()
    x_ext = nc.declare_dram_parameter("x", [STEP, P, FDFULL], fp32, isOutput=False)
    s_ext = nc.declare_dram_parameter("s", [STEP, P, FDFULL], fp32, isOutput=True)
    if USE_PE:
        # [0.5*I | I] stationary matrices for the PE m-update
        ids_ext = nc.declare_dram_parameter("ids", [P, 256], fp32, isOutput=False)

    import contextlib

    ctx = contextlib.ExitStack()
    sb = {}

    def tile(name):
        ap = ctx.enter_context(nc.sbuf_tensor(name, [P, FD], fp32))
        sb[name] = ap
        return ap

    for st in range(NSTREAM):
        for nm in ("X0", "X1", "S0", "S1", "z", "v", "t1", "w", "vt", "m"):
            tile(f"{nm}_{st}")
    ps = {}
    if USE_PE:
        idsb = ctx.enter_context(nc.sbuf_tensor("idsb", [P, 256], fp32))
        for st in range(NSTREAM):
            for b in range(2):
                ps[(st, b)] = ctx.enter_context(
                    nc.psum_tensor(f"ps{st}_{b}", [P, CHUNK], fp32)
                )

    # Per-engine plans: (emit_fn, wait, inc); wait/inc = (sem_name, value)|None.
    plans = {"sync": [], "gpsimd": [], "vector": [], "scalar": [], "tensor": []}
    counts = {"ld": 0, "so": 0, "vec": 0, "act": 0, "gps": 0, "pe": 0}
    mark = {}     # label -> (sem, value) after that op
    last_cp = {}  # (stream, psum bank) -> mark of last ACT copy consuming it

    def emit(engine, fn, wait=None, inc=None, label=None):
        plans[engine].append((fn, wait, inc))
        if inc is not None:
            counts[inc[0]] += inc[1]
            if label is not None:
                mark[label] = (inc[0], counts[inc[0]])

    def xsrc(t, st):
        return x_ext[t][:, FD * st : FD * (st + 1)]

    def sdst(t, st):
        return s_ext[t][:, FD * st : FD * (st + 1)]

    X = lambda t, st: sb[f"X{t % 2}_{st}"]
    S = lambda t, st: sb[f"S{t % 2}_{st}"]

    def dve(label, fn, wait=None):
        emit("vector", fn, wait=wait, inc=("vec", 1), label=label)

    def gps(label, fn, wait=None):
        emit("gpsimd", fn, wait=wait, inc=("gps", 1), label=label)

    def act(label, fn, wait=None):
        emit("scalar", fn, wait=wait, inc=("act", 1), label=label)

    def pe(label, fn, wait=None):
        emit("tensor", fn, wait=wait, inc=("pe", 1), label=label)

    def load(t, st, wait=None):
        emit(
            "sync",
            lambda e, t=t, st=st: e.dma_start(out=X(t, st)[:], in_=xsrc(t, st)),
            wait=wait,
            inc=("ld", 16),
            label=f"ld{t}_{st}",
        )

    def store(t, st):
        emit(
            "scalar",
            lambda e, t=t, st=st: e.dma_start(out=sdst(t, st), in_=S(t, st)[:]),
            wait=mark[f"s{t}_{st}"],
            inc=("so", 16),
            label=f"st{t}_{st}",
        )

    def wait_only(engine, wait):
        emit(engine, lambda e: None, wait=wait)

    # --- prefetch x0, x1 for both streams -------------------------------
    for t in (0, 1):
        for st in range(NSTREAM):
            load(t, st)
    if USE_PE:
        emit(
            "sync",
            lambda e: e.dma_start(out=idsb[:], in_=ids_ext[:]),
            inc=("ld", 16),
            label="ldids",
        )
        # gate the whole PE plan on the identity matrices being resident
        wait_only("tensor", mark["ldids"])

    # ===================== t = 0 =========================================
    # DVE: s0 = (x>=1), vt0 = (x>=1) - x, z0 = vt0 + u0;  ACT: u0 = s0*sigma1
    # m1 aliases S0.
    for st in range(NSTREAM):
        dve(
            f"s0_{st}",
            lambda e, st=st: e.tensor_scalar(
                S(0, st)[:], X(0, st)[:], 1.0, None, Alu.is_ge
            ),
            wait=mark[f"ld0_{st}"],
        )
        dve(
            f"vt0_{st}",
            lambda e, st=st: e.scalar_tensor_tensor(
                sb[f"vt_{st}"][:], X(0, st)[:], 1.0, X(0, st)[:],
                Alu.is_ge, Alu.subtract,
            ),
        )
    for st in range(NSTREAM):
        act(
            f"u0_{st}",
            lambda e, st=st: e.activation(
                sb[f"t1_{st}"][:], S(0, st)[:], Act.Copy, bias=0.0, scale=SIGMA1
            ),
            wait=mark[f"s0_{st}"],
        )
    for st in range(NSTREAM):
        dve(
            f"z0_{st}",
            lambda e, st=st: e.tensor_tensor(
                sb[f"z_{st}"][:], sb[f"vt_{st}"][:], sb[f"t1_{st}"][:], Alu.add
            ),
            wait=mark[f"u0_{st}"],
        )
        store(0, st)

    # prefetch x2 (reuses X0; last X0 consumer at t=0 is vt0)
    for st in range(NSTREAM):
        load(2, st, wait=mark[f"vt0_{st}"])

    # ===================== t = 1, 2 ======================================
    for t in (1, 2):
        for st in range(NSTREAM):
            # v = (z * -0.5) + x
            dve(
                f"v{t}_{st}",
                lambda e, t=t, st=st: e.scalar_tensor_tensor(
                    sb[f"v_{st}"][:], sb[f"z_{st}"][:], -0.5, X(t, st)[:],
                    Alu.mult, Alu.add,
                ),
                wait=mark[f"ld{t}_{st}"],
            )
        for st in range(NSTREAM):
            # ACT: t1 = tanh(0.25 * v)
            act(
                f"t1_{t}_{st}",
                lambda e, st=st: e.activation(
                    sb[f"t1_{st}"][:], sb[f"v_{st}"][:], Act.Tanh, scale=0.25
                ),
                wait=mark[f"v{t}_{st}"],
            )
            # s = (v>=1) into S buffer; vt = (v>=1) - v
            swait = mark[f"st{t-2}_{st}"] if t >= 2 else None
            dve(
                f"s{t}_{st}",
                lambda e, t=t, st=st: e.tensor_scalar(
                    S(t, st)[:], sb[f"v_{st}"][:], 1.0, None, Alu.is_ge
                ),
                wait=swait,
            )
            dve(
                f"vt{t}_{st}",
                lambda e, st=st: e.scalar_tensor_tensor(
                    sb[f"vt_{st}"][:], sb[f"v_{st}"][:], 1.0, sb[f"v_{st}"][:],
                    Alu.is_ge, Alu.subtract,
                ),
            )
        for st in range(NSTREAM):
            mprev = S(0, st) if t == 1 else sb[f"m_{st}"]
            # w = (t1 + 1) * m_prev
            dve(
                f"w{t}_{st}",
                lambda e, st=st, mprev=mprev: e.scalar_tensor_tensor(
                    sb[f"w_{st}"][:], sb[f"t1_{st}"][:], 1.0, mprev[:],
                    Alu.add, Alu.mult,
                ),
                wait=mark[f"t1_{t}_{st}"],
            )
            if not USE_PE:
                # m = (w * 0.5) + s
                dve(
                    f"m{t}_{st}",
                    lambda e, t=t, st=st: e.scalar_tensor_tensor(
                        sb[f"m_{st}"][:], sb[f"w_{st}"][:], 0.5, S(t, st)[:],
                        Alu.mult, Alu.add,
                    ),
                )
        if USE_PE:
            # PE: m = 0.5*w + s per 512-col PSUM chunk; ACT: t2 = tanh(0.5*m)
            # from PSUM + bit-preserving copy of m back to SBUF.
            for st in range(NSTREAM):
                for ci, (c0, csz) in enumerate(CHUNKS):
                    bank = ci % 2
                    pst = ps[(st, bank)]
                    if ci == 0:
                        # vec(w) also transitively implies this bank's previous
                        # ACT copy finished (w waits t1 which follows it).
                        mm1_wait = mark[f"w{t}_{st}"]
                    else:
                        mm1_wait = last_cp.get((st, bank))
                    pe(
                        f"mmA{t}_{st}_c{ci}",
                        lambda e, st=st, c0=c0, csz=csz, pst=pst: e.matmul(
                            pst[:, 0:csz], idsb[:, 0:128],
                            sb[f"w_{st}"][:, c0 : c0 + csz],
                            start=True, stop=False,
                        ),
                        wait=mm1_wait,
                    )
                    pe(
                        f"mmB{t}_{st}_c{ci}",
                        lambda e, t=t, st=st, c0=c0, csz=csz, pst=pst: e.matmul(
                            pst[:, 0:csz], idsb[:, 128:256],
                            S(t, st)[:, c0 : c0 + csz],
                            start=False, stop=True,
                        ),
                    )
                    act(
                        f"t2c{t}_{st}_c{ci}",
                        lambda e, st=st, c0=c0, csz=csz, pst=pst: e.activation(
                            sb[f"t1_{st}"][:, c0 : c0 + csz], pst[:, 0:csz],
                            Act.Tanh, scale=0.5,
                        ),
                        wait=mark[f"mmB{t}_{st}_c{ci}"],
                    )
                    act(
                        f"cp{t}_{st}_c{ci}",
                        lambda e, st=st, c0=c0, csz=csz, pst=pst: e.activation(
                            sb[f"m_{st}"][:, c0 : c0 + csz], pst[:, 0:csz],
                            Act.Copy, bias=0.0, scale=1.0,
                        ),
                    )
                    last_cp[(st, bank)] = mark[f"cp{t}_{st}_c{ci}"]
                mark[f"t2_{t}_{st}"] = mark[f"t2c{t}_{st}_c{len(CHUNKS) - 1}"]
        else:
            for st in range(NSTREAM):
                # ACT: t2 = tanh(0.5 * m) (into t1 buffer)
                act(
                    f"t2_{t}_{st}",
                    lambda e, st=st: e.activation(
                        sb[f"t1_{st}"][:], sb[f"m_{st}"][:], Act.Tanh, scale=0.5
                    ),
                    wait=mark[f"m{t}_{st}"],
                )
        for st in range(NSTREAM):
            # w2 = (t2 + 1) * s ; z = (w2 * 0.5) + vt
            dve(
                f"w2{t}_{st}",
                lambda e, t=t, st=st: e.scalar_tensor_tensor(
                    sb[f"w_{st}"][:], sb[f"t1_{st}"][:], 1.0, S(t, st)[:],
                    Alu.add, Alu.mult,
                ),
                wait=mark[f"t2_{t}_{st}"],
            )
            dve(
                f"z{t}_{st}",
                lambda e, st=st: e.scalar_tensor_tensor(
                    sb[f"z_{st}"][:], sb[f"w_{st}"][:], 0.5, sb[f"vt_{st}"][:],
                    Alu.mult, Alu.add,
                ),
            )
            store(t, st)
        if t == 1:
            for st in range(NSTREAM):
                # x3 reuses X1; last X1 consumer was v1
                load(3, st, wait=mark[f"v1_{st}"])

    # ===================== t = 3 =========================================
    for st in range(NSTREAM):
        dve(
            f"v3_{st}",
            lambda e, st=st: e.scalar_tensor_tensor(
                sb[f"v_{st}"][:], sb[f"z_{st}"][:], -0.5, X(3, st)[:],
                Alu.mult, Alu.add,
            ),
            wait=mark[f"ld3_{st}"],
        )
    for st in range(NSTREAM):
        dve(
            f"s3_{st}",
            lambda e, st=st: e.tensor_scalar(
                S(3, st)[:], sb[f"v_{st}"][:], 1.0, None, Alu.is_ge
            ),
            wait=mark[f"st1_{st}"],
        )
        store(3, st)

    # ---------------------------------------------------------------------
    final_so = counts["so"]
    with (
        nc.Block() as block,
        nc.semaphore("ld") as ld_sem,
        nc.semaphore("so") as so_sem,
        nc.semaphore("vec") as vec_sem,
        nc.semaphore("act") as act_sem,
        nc.semaphore("gps") as gps_sem,
        nc.semaphore("pe") as pe_sem,
    ):
        sems = {"ld": ld_sem, "so": so_sem, "vec": vec_sem, "act": act_sem,
                "gps": gps_sem, "pe": pe_sem}

        def run_plan(engine_handle, plan, final_wait=None):
            for fn, wait, inc in plan:
                ins = fn(engine_handle)
                if ins is None:
                    assert wait is not None and inc is None
                    engine_handle.wait_ge(sems[wait[0]], wait[1])
                    continue
                if wait is not None:
                    ins._wait_ge(sems[wait[0]], wait[1])
                if inc is not None:
                    ins.then_inc(sems[inc[0]], inc[1])
            if final_wait is not None:
                engine_handle.wait_ge(sems[final_wait[0]], final_wait[1])

        @block.sync
        def _(e):
            run_plan(e, plans["sync"])

        @block.tensor
        def _(e):
            run_plan(e, plans["tensor"])

        @block.gpsimd
        def _(e):
            run_plan(e, plans["gpsimd"])

        @block.vector
        def _(e):
            run_plan(e, plans["vector"])

        @block.scalar
        def _(e):
            run_plan(e, plans["scalar"], final_wait=("so", final_so))

    ctx.close()
    return nc


def _get_program():
    if "nc" not in _CACHE:
        _ensure_axon_hooks()
        _CACHE["nc"] = build_bass()
    return _CACHE["nc"]


def shard_inputs(x_seq):
    """x_seq [(t*b), n, c] -> per-core [STEP, P, FDFULL] contiguous blocks."""
    xt = np.ascontiguousarray(x_seq).reshape(STEP, B, N * C)
    if USE_PE:
        ids = np.zeros((P, 256), dtype=np.float32)
        ids[:, 0:128] = 0.5 * np.eye(P, dtype=np.float32)
        ids[:, 128:256] = np.eye(P, dtype=np.float32)
    maps = []
    for k in range(NCORES):
        blk = xt[:, k * BPC : (k + 1) * BPC, :].reshape(STEP, P, FDFULL)
        m = {"x": np.ascontiguousarray(blk)}
        if USE_PE:
            m["ids"] = ids.copy()
        maps.append(m)
    return maps


def unshard_outputs(results):
    """Per-core [STEP, P, FDFULL] spike blocks -> [(t*b), n, c]."""
    out = np.empty((STEP, B, N * C), dtype=np.float32)
    for k in range(NCORES):
        blk = results[k]["s"].reshape(STEP, BPC, N * C)
        out[:, k * BPC : (k + 1) * BPC, :] = blk
    return out.reshape(STEP * B, N, C)


def kernel(x_seq, step, _trace=False):
    assert int(step) == STEP
    assert x_seq.shape == (STEP * B, N, C)
    x_seq = np.asarray(x_seq, dtype=np.float32)

    from concourse.bass_utils import run_bass_kernel_spmd

    nc = _get_program()
    in_maps = shard_inputs(x_seq)
    res = run_bass_kernel_spmd(nc, in_maps, list(range(NCORES)), trace=_trace)
    out = unshard_outputs(res.results)
    if _trace:
        return out, res
    return out


# revision 26
# speedup vs baseline: 1.1760x; 1.1760x over previous
"""ComplementaryLIFNeuron on 8 Trainium2 NeuronCores (Bass, raw engine blocks).

Reference recurrence (per time step t, elementwise over [b, n, c]):
    v = v * 0.5 + x
    p = sigmoid(v / 2)          # 0.5 + 0.5*tanh(v/4)
    m = m * p
    s = (v >= 1)
    m = m + s
    q = sigmoid(m)              # 0.5 + 0.5*tanh(m/2)
    v = (v - s) - s * q
Output is s for each step, shape [(t*b), n, c].

Sharding: data-parallel over batch b=32 -> 4 rows per core; each (t, core)
block is a contiguous [4, 196*768] = [128, 4704] fp32 chunk, split into
2 interleaved column streams of 2352.

Exactness strategy (bit-identical to XLA fp32 except the tanh tables):
    m*p == 0.5 * ((tanh+1)*m)   (pow2 scaling commutes with RNE)
    track z = -((v-s) - s*q)    (negation symmetry of RNE), so
    s - v, + s*q and the next charge v' = (-0.5)*z + x all match exactly.
    t=0: v0 = x, s0 = (x>=1), m1 = s0, q0 = s0 * sigma(1) with sigma(1)
    hardcoded to XLA's fp32 bit pattern. t=3: only v3, s3 computed.

Engine split (per NeuronCore):
    DVE    : all 2-input STT/TT chain ops (never touch the shared SBUF port)
    GPSIMD : spike compare (tensor_scalar is_ge) + vt=(v>=1)-v STT
    ACT    : tanh's, sigma(1) scaling, spike store DMA issue (HWDGE)
    SYNC   : input load DMA issue (HWDGE)
"""

import sys
import types
import numpy as np

STEP = 4
B = 32
N = 196
C = 768
NCORES = 8
BPC = B // NCORES            # batch rows per core = 4
PELEM = BPC * N * C          # elements per (t, core) block = 602112
P = 128                      # SBUF partitions
FDFULL = PELEM // P          # 4704 free-dim columns per (t, core)
NSTREAM = 2                  # independent column streams (interleaved)
FD = FDFULL // NSTREAM       # 2352 columns per stream tile

SIGMA1 = float(np.uint32(0x3F3B26A8).view(np.float32))  # XLA fp32 sigmoid(1.0)

USE_PE = False        # TensorEngine m-update: bit-exact but net slower
CHUNK = 512           # PSUM bank = 512 fp32 columns
CHUNKS = [(c, min(CHUNK, FD - c)) for c in range(0, FD, CHUNK)]

_CACHE = {}


def _ensure_axon_hooks():
    """bass_utils' trace path imports antenv.axon_hooks, absent in this image.

    Recreate the module and register the ctypes NTFF hook that
    trn_agent_boot would have installed if the module existed.
    """
    import antenv

    if "antenv.axon_hooks" not in sys.modules:
        m = types.ModuleType("antenv.axon_hooks")
        hook = [None]
        m.set_axon_ntff_profile_hook = lambda h: hook.__setitem__(0, h)
        m.get_axon_ntff_profile_hook = lambda: hook[0]
        sys.modules["antenv.axon_hooks"] = m
        antenv.axon_hooks = m
        try:
            from trn_agent_boot.trn_boot import _ntff_profile_via_ctypes

            h = _ntff_profile_via_ctypes("/opt/axon/libaxon_pjrt.so")
            if h is not None:
                m.set_axon_ntff_profile_hook(h)
        except Exception:
            pass


def build_bass():
    """Build the per-core SPMD Bass program."""
    from concourse import bass
    import concourse.mybir as mybir

    fp32 = mybir.dt.float32
    Alu = mybir.AluOpType
    Act = mybir.ActivationFunctionType

    nc = bass.Bass()
    x_ext = nc.declare_dram_parameter("x", [STEP, P, FDFULL], fp32, isOutput=False)
    s_ext = nc.declare_dram_parameter("s", [STEP, P, FDFULL], mybir.dt.uint8, isOutput=True)
    if USE_PE:
        # [0.5*I | I] stationary matrices for the PE m-update
        ids_ext = nc.declare_dram_parameter("ids", [P, 256], fp32, isOutput=False)

    import contextlib

    ctx = contextlib.ExitStack()
    sb = {}

    def tile(name):
        ap = ctx.enter_context(nc.sbuf_tensor(name, [P, FD], fp32))
        sb[name] = ap
        return ap

    for st in range(NSTREAM):
        for nm in ("X0", "X1", "z", "v", "t1", "w", "vt", "m"):
            tile(f"{nm}_{st}")
        for nm in ("S0", "S1"):
            ap = ctx.enter_context(
                nc.sbuf_tensor(f"{nm}_{st}", [P, FD], mybir.dt.uint8)
            )
            sb[f"{nm}_{st}"] = ap
    ps = {}
    if USE_PE:
        idsb = ctx.enter_context(nc.sbuf_tensor("idsb", [P, 256], fp32))
        for st in range(NSTREAM):
            for b in range(2):
                ps[(st, b)] = ctx.enter_context(
                    nc.psum_tensor(f"ps{st}_{b}", [P, CHUNK], fp32)
                )

    # Per-engine plans: (emit_fn, wait, inc); wait/inc = (sem_name, value)|None.
    plans = {"sync": [], "gpsimd": [], "vector": [], "scalar": [], "tensor": []}
    counts = {"ld": 0, "so": 0, "vec": 0, "act": 0, "gps": 0, "pe": 0}
    mark = {}     # label -> (sem, value) after that op
    last_cp = {}  # (stream, psum bank) -> mark of last ACT copy consuming it

    def emit(engine, fn, wait=None, inc=None, label=None):
        plans[engine].append((fn, wait, inc))
        if inc is not None:
            counts[inc[0]] += inc[1]
            if label is not None:
                mark[label] = (inc[0], counts[inc[0]])

    def xsrc(t, st):
        return x_ext[t][:, FD * st : FD * (st + 1)]

    def sdst(t, st):
        return s_ext[t][:, FD * st : FD * (st + 1)]

    X = lambda t, st: sb[f"X{t % 2}_{st}"]
    S = lambda t, st: sb[f"S{t % 2}_{st}"]

    def dve(label, fn, wait=None):
        emit("vector", fn, wait=wait, inc=("vec", 1), label=label)

    def gps(label, fn, wait=None):
        emit("gpsimd", fn, wait=wait, inc=("gps", 1), label=label)

    def act(label, fn, wait=None):
        emit("scalar", fn, wait=wait, inc=("act", 1), label=label)

    def pe(label, fn, wait=None):
        emit("tensor", fn, wait=wait, inc=("pe", 1), label=label)

    def load(t, st, wait=None):
        emit(
            "sync",
            lambda e, t=t, st=st: e.dma_start(out=X(t, st)[:], in_=xsrc(t, st)),
            wait=wait,
            inc=("ld", 16),
            label=f"ld{t}_{st}",
        )

    def store(t, st):
        emit(
            "scalar",
            lambda e, t=t, st=st: e.dma_start(out=sdst(t, st), in_=S(t, st)[:]),
            wait=mark[f"s{t}_{st}"],
            inc=("so", 16),
            label=f"st{t}_{st}",
        )

    def wait_only(engine, wait):
        emit(engine, lambda e: None, wait=wait)

    # --- prefetch x0, x1 for both streams -------------------------------
    for t in (0, 1):
        for st in range(NSTREAM):
            load(t, st)
    if USE_PE:
        emit(
            "sync",
            lambda e: e.dma_start(out=idsb[:], in_=ids_ext[:]),
            inc=("ld", 16),
            label="ldids",
        )
        # gate the whole PE plan on the identity matrices being resident
        wait_only("tensor", mark["ldids"])

    # ===================== t = 0 =========================================
    # DVE: s0 = (x>=1), vt0 = (x>=1) - x, z0 = vt0 + u0;  ACT: u0 = s0*sigma1
    # m1 aliases S0.
    for st in range(NSTREAM):
        dve(
            f"s0_{st}",
            lambda e, st=st: e.tensor_scalar(
                S(0, st)[:], X(0, st)[:], 1.0, None, Alu.is_ge
            ),
            wait=mark[f"ld0_{st}"],
        )
        dve(
            f"vt0_{st}",
            lambda e, st=st: e.scalar_tensor_tensor(
                sb[f"vt_{st}"][:], X(0, st)[:], 1.0, X(0, st)[:],
                Alu.is_ge, Alu.subtract,
            ),
        )
    for st in range(NSTREAM):
        dve(
            f"z0_{st}",
            lambda e, st=st: e.scalar_tensor_tensor(
                sb[f"z_{st}"][:], S(0, st)[:], SIGMA1, sb[f"vt_{st}"][:],
                Alu.mult, Alu.add,
            ),
        )
        store(0, st)

    # prefetch x2 (reuses X0; last X0 consumer at t=0 is vt0)
    for st in range(NSTREAM):
        load(2, st, wait=mark[f"vt0_{st}"])

    # ===================== t = 1, 2 ======================================
    for t in (1, 2):
        for st in range(NSTREAM):
            # v = (z * -0.5) + x
            dve(
                f"v{t}_{st}",
                lambda e, t=t, st=st: e.scalar_tensor_tensor(
                    sb[f"v_{st}"][:], sb[f"z_{st}"][:], -0.5, X(t, st)[:],
                    Alu.mult, Alu.add,
                ),
                wait=mark[f"ld{t}_{st}"],
            )
        for st in range(NSTREAM):
            # ACT: t1 = tanh(0.25 * v)
            act(
                f"t1_{t}_{st}",
                lambda e, st=st: e.activation(
                    sb[f"t1_{st}"][:], sb[f"v_{st}"][:], Act.Tanh, scale=0.25
                ),
                wait=mark[f"v{t}_{st}"],
            )
            # s = (v>=1) into S buffer; vt = (v>=1) - v
            swait = mark[f"st{t-2}_{st}"] if t >= 2 else None
            dve(
                f"s{t}_{st}",
                lambda e, t=t, st=st: e.tensor_scalar(
                    S(t, st)[:], sb[f"v_{st}"][:], 1.0, None, Alu.is_ge
                ),
                wait=swait,
            )
            dve(
                f"vt{t}_{st}",
                lambda e, st=st: e.scalar_tensor_tensor(
                    sb[f"vt_{st}"][:], sb[f"v_{st}"][:], 1.0, sb[f"v_{st}"][:],
                    Alu.is_ge, Alu.subtract,
                ),
            )
        for st in range(NSTREAM):
            mprev = S(0, st) if t == 1 else sb[f"m_{st}"]
            # w = (t1 + 1) * m_prev
            dve(
                f"w{t}_{st}",
                lambda e, st=st, mprev=mprev: e.scalar_tensor_tensor(
                    sb[f"w_{st}"][:], sb[f"t1_{st}"][:], 1.0, mprev[:],
                    Alu.add, Alu.mult,
                ),
                wait=mark[f"t1_{t}_{st}"],
            )
            if not USE_PE:
                # m = (w * 0.5) + s
                dve(
                    f"m{t}_{st}",
                    lambda e, t=t, st=st: e.scalar_tensor_tensor(
                        sb[f"m_{st}"][:], sb[f"w_{st}"][:], 0.5, S(t, st)[:],
                        Alu.mult, Alu.add,
                    ),
                )
        if USE_PE:
            # PE: m = 0.5*w + s per 512-col PSUM chunk; ACT: t2 = tanh(0.5*m)
            # from PSUM + bit-preserving copy of m back to SBUF.
            for st in range(NSTREAM):
                for ci, (c0, csz) in enumerate(CHUNKS):
                    bank = ci % 2
                    pst = ps[(st, bank)]
                    if ci == 0:
                        # vec(w) also transitively implies this bank's previous
                        # ACT copy finished (w waits t1 which follows it).
                        mm1_wait = mark[f"w{t}_{st}"]
                    else:
                        mm1_wait = last_cp.get((st, bank))
                    pe(
                        f"mmA{t}_{st}_c{ci}",
                        lambda e, st=st, c0=c0, csz=csz, pst=pst: e.matmul(
                            pst[:, 0:csz], idsb[:, 0:128],
                            sb[f"w_{st}"][:, c0 : c0 + csz],
                            start=True, stop=False,
                        ),
                        wait=mm1_wait,
                    )
                    pe(
                        f"mmB{t}_{st}_c{ci}",
                        lambda e, t=t, st=st, c0=c0, csz=csz, pst=pst: e.matmul(
                            pst[:, 0:csz], idsb[:, 128:256],
                            S(t, st)[:, c0 : c0 + csz],
                            start=False, stop=True,
                        ),
                    )
                    act(
                        f"t2c{t}_{st}_c{ci}",
                        lambda e, st=st, c0=c0, csz=csz, pst=pst: e.activation(
                            sb[f"t1_{st}"][:, c0 : c0 + csz], pst[:, 0:csz],
                            Act.Tanh, scale=0.5,
                        ),
                        wait=mark[f"mmB{t}_{st}_c{ci}"],
                    )
                    act(
                        f"cp{t}_{st}_c{ci}",
                        lambda e, st=st, c0=c0, csz=csz, pst=pst: e.activation(
                            sb[f"m_{st}"][:, c0 : c0 + csz], pst[:, 0:csz],
                            Act.Copy, bias=0.0, scale=1.0,
                        ),
                    )
                    last_cp[(st, bank)] = mark[f"cp{t}_{st}_c{ci}"]
                mark[f"t2_{t}_{st}"] = mark[f"t2c{t}_{st}_c{len(CHUNKS) - 1}"]
        else:
            for st in range(NSTREAM):
                # ACT: t2 = tanh(0.5 * m) (into t1 buffer)
                act(
                    f"t2_{t}_{st}",
                    lambda e, st=st: e.activation(
                        sb[f"t1_{st}"][:], sb[f"m_{st}"][:], Act.Tanh, scale=0.5
                    ),
                    wait=mark[f"m{t}_{st}"],
                )
        for st in range(NSTREAM):
            # w2 = (t2 + 1) * s ; z = (w2 * 0.5) + vt
            dve(
                f"w2{t}_{st}",
                lambda e, t=t, st=st: e.scalar_tensor_tensor(
                    sb[f"w_{st}"][:], sb[f"t1_{st}"][:], 1.0, S(t, st)[:],
                    Alu.add, Alu.mult,
                ),
                wait=mark[f"t2_{t}_{st}"],
            )
            dve(
                f"z{t}_{st}",
                lambda e, st=st: e.scalar_tensor_tensor(
                    sb[f"z_{st}"][:], sb[f"w_{st}"][:], 0.5, sb[f"vt_{st}"][:],
                    Alu.mult, Alu.add,
                ),
            )
            store(t, st)
        if t == 1:
            for st in range(NSTREAM):
                # x3 reuses X1; last X1 consumer was v1
                load(3, st, wait=mark[f"v1_{st}"])

    # ===================== t = 3 =========================================
    for st in range(NSTREAM):
        dve(
            f"v3_{st}",
            lambda e, st=st: e.scalar_tensor_tensor(
                sb[f"v_{st}"][:], sb[f"z_{st}"][:], -0.5, X(3, st)[:],
                Alu.mult, Alu.add,
            ),
            wait=mark[f"ld3_{st}"],
        )
    # split the final spike tiles so their stores drain during the epilogue
    half = FD // 2
    for st in range(NSTREAM):
        for h, (h0, hsz) in enumerate(((0, half), (half, FD - half))):
            dve(
                f"s3_{st}_h{h}",
                lambda e, st=st, h0=h0, hsz=hsz: e.tensor_scalar(
                    S(3, st)[:, h0 : h0 + hsz], sb[f"v_{st}"][:, h0 : h0 + hsz],
                    1.0, None, Alu.is_ge,
                ),
                wait=mark[f"st1_{st}"] if h == 0 else None,
            )
            emit(
                "scalar",
                lambda e, st=st, h0=h0, hsz=hsz: e.dma_start(
                    out=s_ext[3][:, FD * st + h0 : FD * st + h0 + hsz],
                    in_=S(3, st)[:, h0 : h0 + hsz],
                ),
                wait=mark[f"s3_{st}_h{h}"],
                inc=("so", 16),
                label=f"st3_{st}_h{h}",
            )

    # ---------------------------------------------------------------------
    final_so = counts["so"]
    with (
        nc.Block() as block,
        nc.semaphore("ld") as ld_sem,
        nc.semaphore("so") as so_sem,
        nc.semaphore("vec") as vec_sem,
        nc.semaphore("act") as act_sem,
        nc.semaphore("gps") as gps_sem,
        nc.semaphore("pe") as pe_sem,
    ):
        sems = {"ld": ld_sem, "so": so_sem, "vec": vec_sem, "act": act_sem,
                "gps": gps_sem, "pe": pe_sem}

        def run_plan(engine_handle, plan, final_wait=None):
            for fn, wait, inc in plan:
                ins = fn(engine_handle)
                if ins is None:
                    assert wait is not None and inc is None
                    engine_handle.wait_ge(sems[wait[0]], wait[1])
                    continue
                if wait is not None:
                    ins._wait_ge(sems[wait[0]], wait[1])
                if inc is not None:
                    ins.then_inc(sems[inc[0]], inc[1])
            if final_wait is not None:
                engine_handle.wait_ge(sems[final_wait[0]], final_wait[1])

        @block.sync
        def _(e):
            run_plan(e, plans["sync"])

        @block.tensor
        def _(e):
            run_plan(e, plans["tensor"])

        @block.gpsimd
        def _(e):
            run_plan(e, plans["gpsimd"])

        @block.vector
        def _(e):
            run_plan(e, plans["vector"])

        @block.scalar
        def _(e):
            run_plan(e, plans["scalar"], final_wait=("so", final_so))

    ctx.close()
    return nc


def _get_program():
    if "nc" not in _CACHE:
        _ensure_axon_hooks()
        _CACHE["nc"] = build_bass()
    return _CACHE["nc"]


def shard_inputs(x_seq):
    """x_seq [(t*b), n, c] -> per-core [STEP, P, FDFULL] contiguous blocks."""
    xt = np.ascontiguousarray(x_seq).reshape(STEP, B, N * C)
    if USE_PE:
        ids = np.zeros((P, 256), dtype=np.float32)
        ids[:, 0:128] = 0.5 * np.eye(P, dtype=np.float32)
        ids[:, 128:256] = np.eye(P, dtype=np.float32)
    maps = []
    for k in range(NCORES):
        blk = xt[:, k * BPC : (k + 1) * BPC, :].reshape(STEP, P, FDFULL)
        m = {"x": np.ascontiguousarray(blk)}
        if USE_PE:
            m["ids"] = ids.copy()
        maps.append(m)
    return maps


def unshard_outputs(results):
    """Per-core [STEP, P, FDFULL] spike blocks -> [(t*b), n, c]."""
    out = np.empty((STEP, B, N * C), dtype=np.float32)
    for k in range(NCORES):
        blk = results[k]["s"].reshape(STEP, BPC, N * C)
        out[:, k * BPC : (k + 1) * BPC, :] = blk
    return out.reshape(STEP * B, N, C)


def kernel(x_seq, step, _trace=False):
    assert int(step) == STEP
    assert x_seq.shape == (STEP * B, N, C)
    x_seq = np.asarray(x_seq, dtype=np.float32)

    from concourse.bass_utils import run_bass_kernel_spmd

    nc = _get_program()
    in_maps = shard_inputs(x_seq)
    res = run_bass_kernel_spmd(nc, in_maps, list(range(NCORES)), trace=_trace)
    out = unshard_outputs(res.results)
    if _trace:
        return out, res
    return out


# revision 27
# speedup vs baseline: 1.1896x; 1.0115x over previous
"""ComplementaryLIFNeuron on 8 Trainium2 NeuronCores (Bass, raw engine blocks).

Reference recurrence (per time step t, elementwise over [b, n, c]):
    v = v * 0.5 + x
    p = sigmoid(v / 2)          # 0.5 + 0.5*tanh(v/4)
    m = m * p
    s = (v >= 1)
    m = m + s
    q = sigmoid(m)              # 0.5 + 0.5*tanh(m/2)
    v = (v - s) - s * q
Output is s for each step, shape [(t*b), n, c].

Sharding: data-parallel over batch b=32 -> 4 rows per core; each (t, core)
block is a contiguous [4, 196*768] = [128, 4704] fp32 chunk, split into
2 interleaved column streams of 2352.

Exactness strategy (bit-identical to XLA fp32 except the tanh tables):
    m*p == 0.5 * ((tanh+1)*m)   (pow2 scaling commutes with RNE)
    track z = -((v-s) - s*q)    (negation symmetry of RNE), so
    s - v, + s*q and the next charge v' = (-0.5)*z + x all match exactly.
    t=0: v0 = x, s0 = (x>=1), m1 = s0, q0 = s0 * sigma(1) with sigma(1)
    hardcoded to XLA's fp32 bit pattern. t=3: only v3, s3 computed.

Engine split (per NeuronCore):
    DVE    : all 2-input STT/TT chain ops (never touch the shared SBUF port)
    GPSIMD : spike compare (tensor_scalar is_ge) + vt=(v>=1)-v STT
    ACT    : tanh's, sigma(1) scaling, spike store DMA issue (HWDGE)
    SYNC   : input load DMA issue (HWDGE)
"""

import sys
import types
import numpy as np

STEP = 4
B = 32
N = 196
C = 768
NCORES = 8
BPC = B // NCORES            # batch rows per core = 4
PELEM = BPC * N * C          # elements per (t, core) block = 602112
P = 128                      # SBUF partitions
FDFULL = PELEM // P          # 4704 free-dim columns per (t, core)
NSTREAM = 2                  # independent column streams (interleaved)
FD = FDFULL // NSTREAM       # 2352 columns per stream tile

SIGMA1 = float(np.uint32(0x3F3B26A8).view(np.float32))  # XLA fp32 sigmoid(1.0)

USE_PE = False        # TensorEngine m-update: bit-exact but net slower
CHUNK = 512           # PSUM bank = 512 fp32 columns
CHUNKS = [(c, min(CHUNK, FD - c)) for c in range(0, FD, CHUNK)]

_CACHE = {}


def _ensure_axon_hooks():
    """bass_utils' trace path imports antenv.axon_hooks, absent in this image.

    Recreate the module and register the ctypes NTFF hook that
    trn_agent_boot would have installed if the module existed.
    """
    import antenv

    if "antenv.axon_hooks" not in sys.modules:
        m = types.ModuleType("antenv.axon_hooks")
        hook = [None]
        m.set_axon_ntff_profile_hook = lambda h: hook.__setitem__(0, h)
        m.get_axon_ntff_profile_hook = lambda: hook[0]
        sys.modules["antenv.axon_hooks"] = m
        antenv.axon_hooks = m
        try:
            from trn_agent_boot.trn_boot import _ntff_profile_via_ctypes

            h = _ntff_profile_via_ctypes("/opt/axon/libaxon_pjrt.so")
            if h is not None:
                m.set_axon_ntff_profile_hook(h)
        except Exception:
            pass


def build_bass():
    """Build the per-core SPMD Bass program."""
    from concourse import bass
    import concourse.mybir as mybir

    fp32 = mybir.dt.float32
    Alu = mybir.AluOpType
    Act = mybir.ActivationFunctionType

    nc = bass.Bass()
    x_ext = nc.declare_dram_parameter("x", [STEP, P, FDFULL], fp32, isOutput=False)
    s_ext = nc.declare_dram_parameter("s", [STEP, P, FDFULL], mybir.dt.uint8, isOutput=True)
    if USE_PE:
        # [0.5*I | I] stationary matrices for the PE m-update
        ids_ext = nc.declare_dram_parameter("ids", [P, 256], fp32, isOutput=False)

    import contextlib

    ctx = contextlib.ExitStack()
    sb = {}

    def tile(name):
        ap = ctx.enter_context(nc.sbuf_tensor(name, [P, FD], fp32))
        sb[name] = ap
        return ap

    for st in range(NSTREAM):
        for nm in ("X0", "X1", "z", "v", "t1", "w", "vt", "m"):
            tile(f"{nm}_{st}")
        for nm in ("S0", "S1"):
            ap = ctx.enter_context(
                nc.sbuf_tensor(f"{nm}_{st}", [P, FD], mybir.dt.uint8)
            )
            sb[f"{nm}_{st}"] = ap
    ps = {}
    if USE_PE:
        idsb = ctx.enter_context(nc.sbuf_tensor("idsb", [P, 256], fp32))
        for st in range(NSTREAM):
            for b in range(2):
                ps[(st, b)] = ctx.enter_context(
                    nc.psum_tensor(f"ps{st}_{b}", [P, CHUNK], fp32)
                )

    # Per-engine plans: (emit_fn, wait, inc); wait/inc = (sem_name, value)|None.
    plans = {"sync": [], "gpsimd": [], "vector": [], "scalar": [], "tensor": []}
    counts = {"ld": 0, "so": 0, "vec": 0, "act": 0, "gps": 0, "pe": 0}
    mark = {}     # label -> (sem, value) after that op
    last_cp = {}  # (stream, psum bank) -> mark of last ACT copy consuming it

    def emit(engine, fn, wait=None, inc=None, label=None):
        plans[engine].append((fn, wait, inc))
        if inc is not None:
            counts[inc[0]] += inc[1]
            if label is not None:
                mark[label] = (inc[0], counts[inc[0]])

    def xsrc(t, st):
        return x_ext[t][:, FD * st : FD * (st + 1)]

    def sdst(t, st):
        return s_ext[t][:, FD * st : FD * (st + 1)]

    X = lambda t, st: sb[f"X{t % 2}_{st}"]
    S = lambda t, st: sb[f"S{t % 2}_{st}"]

    def dve(label, fn, wait=None):
        emit("vector", fn, wait=wait, inc=("vec", 1), label=label)

    def gps(label, fn, wait=None):
        emit("gpsimd", fn, wait=wait, inc=("gps", 1), label=label)

    def act(label, fn, wait=None):
        emit("scalar", fn, wait=wait, inc=("act", 1), label=label)

    def pe(label, fn, wait=None):
        emit("tensor", fn, wait=wait, inc=("pe", 1), label=label)

    def load(t, st, wait=None):
        emit(
            "sync",
            lambda e, t=t, st=st: e.dma_start(out=X(t, st)[:], in_=xsrc(t, st)),
            wait=wait,
            inc=("ld", 16),
            label=f"ld{t}_{st}",
        )

    def store(t, st):
        emit(
            "scalar",
            lambda e, t=t, st=st: e.dma_start(out=sdst(t, st), in_=S(t, st)[:]),
            wait=mark[f"s{t}_{st}"],
            inc=("so", 16),
            label=f"st{t}_{st}",
        )

    def wait_only(engine, wait):
        emit(engine, lambda e: None, wait=wait)

    # --- prefetch x0, x1 for both streams -------------------------------
    QS = 588  # early-start slice of stream 0's first tile
    emit(
        "sync",
        lambda e: e.dma_start(out=X(0, 0)[:, 0:QS], in_=xsrc(0, 0)[:, 0:QS]),
        inc=("ld", 16),
        label="ld0_0a",
    )
    emit(
        "sync",
        lambda e: e.dma_start(out=X(0, 0)[:, QS:FD], in_=xsrc(0, 0)[:, QS:FD]),
        inc=("ld", 16),
        label="ld0_0",
    )
    load(0, 1)
    for st in range(NSTREAM):
        load(1, st)
    if USE_PE:
        emit(
            "sync",
            lambda e: e.dma_start(out=idsb[:], in_=ids_ext[:]),
            inc=("ld", 16),
            label="ldids",
        )
        # gate the whole PE plan on the identity matrices being resident
        wait_only("tensor", mark["ldids"])

    # ===================== t = 0 =========================================
    # DVE: s0 = (x>=1), vt0 = (x>=1) - x, z0 = vt0 + u0;  ACT: u0 = s0*sigma1
    # m1 aliases S0.
    for st in range(NSTREAM):
        if st == 0:
            dve(
                "s0_0a",
                lambda e: e.tensor_scalar(
                    S(0, 0)[:, 0:QS], X(0, 0)[:, 0:QS], 1.0, None, Alu.is_ge
                ),
                wait=mark["ld0_0a"],
            )
            dve(
                "vt0_0a",
                lambda e: e.scalar_tensor_tensor(
                    sb["vt_0"][:, 0:QS], X(0, 0)[:, 0:QS], 1.0,
                    X(0, 0)[:, 0:QS], Alu.is_ge, Alu.subtract,
                ),
            )
            dve(
                "s0_0",
                lambda e: e.tensor_scalar(
                    S(0, 0)[:, QS:FD], X(0, 0)[:, QS:FD], 1.0, None, Alu.is_ge
                ),
                wait=mark["ld0_0"],
            )
            dve(
                "vt0_0",
                lambda e: e.scalar_tensor_tensor(
                    sb["vt_0"][:, QS:FD], X(0, 0)[:, QS:FD], 1.0,
                    X(0, 0)[:, QS:FD], Alu.is_ge, Alu.subtract,
                ),
            )
        else:
            dve(
                f"s0_{st}",
                lambda e, st=st: e.tensor_scalar(
                    S(0, st)[:], X(0, st)[:], 1.0, None, Alu.is_ge
                ),
                wait=mark[f"ld0_{st}"],
            )
            dve(
                f"vt0_{st}",
                lambda e, st=st: e.scalar_tensor_tensor(
                    sb[f"vt_{st}"][:], X(0, st)[:], 1.0, X(0, st)[:],
                    Alu.is_ge, Alu.subtract,
                ),
            )
    for st in range(NSTREAM):
        dve(
            f"z0_{st}",
            lambda e, st=st: e.scalar_tensor_tensor(
                sb[f"z_{st}"][:], S(0, st)[:], SIGMA1, sb[f"vt_{st}"][:],
                Alu.mult, Alu.add,
            ),
        )
        store(0, st)

    # prefetch x2 (reuses X0; last X0 consumer at t=0 is vt0)
    for st in range(NSTREAM):
        load(2, st, wait=mark[f"vt0_{st}"])

    # ===================== t = 1, 2 ======================================
    for t in (1, 2):
        for st in range(NSTREAM):
            # v = (z * -0.5) + x
            dve(
                f"v{t}_{st}",
                lambda e, t=t, st=st: e.scalar_tensor_tensor(
                    sb[f"v_{st}"][:], sb[f"z_{st}"][:], -0.5, X(t, st)[:],
                    Alu.mult, Alu.add,
                ),
                wait=mark[f"ld{t}_{st}"],
            )
        for st in range(NSTREAM):
            # ACT: t1 = tanh(0.25 * v)
            act(
                f"t1_{t}_{st}",
                lambda e, st=st: e.activation(
                    sb[f"t1_{st}"][:], sb[f"v_{st}"][:], Act.Tanh, scale=0.25
                ),
                wait=mark[f"v{t}_{st}"],
            )
            # s = (v>=1) into S buffer; vt = (v>=1) - v
            swait = mark[f"st{t-2}_{st}"] if t >= 2 else None
            dve(
                f"s{t}_{st}",
                lambda e, t=t, st=st: e.tensor_scalar(
                    S(t, st)[:], sb[f"v_{st}"][:], 1.0, None, Alu.is_ge
                ),
                wait=swait,
            )
            dve(
                f"vt{t}_{st}",
                lambda e, st=st: e.scalar_tensor_tensor(
                    sb[f"vt_{st}"][:], sb[f"v_{st}"][:], 1.0, sb[f"v_{st}"][:],
                    Alu.is_ge, Alu.subtract,
                ),
            )
        for st in range(NSTREAM):
            mprev = S(0, st) if t == 1 else sb[f"m_{st}"]
            # w = (t1 + 1) * m_prev
            dve(
                f"w{t}_{st}",
                lambda e, st=st, mprev=mprev: e.scalar_tensor_tensor(
                    sb[f"w_{st}"][:], sb[f"t1_{st}"][:], 1.0, mprev[:],
                    Alu.add, Alu.mult,
                ),
                wait=mark[f"t1_{t}_{st}"],
            )
            if not USE_PE:
                # m = (w * 0.5) + s
                dve(
                    f"m{t}_{st}",
                    lambda e, t=t, st=st: e.scalar_tensor_tensor(
                        sb[f"m_{st}"][:], sb[f"w_{st}"][:], 0.5, S(t, st)[:],
                        Alu.mult, Alu.add,
                    ),
                )
        if USE_PE:
            # PE: m = 0.5*w + s per 512-col PSUM chunk; ACT: t2 = tanh(0.5*m)
            # from PSUM + bit-preserving copy of m back to SBUF.
            for st in range(NSTREAM):
                for ci, (c0, csz) in enumerate(CHUNKS):
                    bank = ci % 2
                    pst = ps[(st, bank)]
                    if ci == 0:
                        # vec(w) also transitively implies this bank's previous
                        # ACT copy finished (w waits t1 which follows it).
                        mm1_wait = mark[f"w{t}_{st}"]
                    else:
                        mm1_wait = last_cp.get((st, bank))
                    pe(
                        f"mmA{t}_{st}_c{ci}",
                        lambda e, st=st, c0=c0, csz=csz, pst=pst: e.matmul(
                            pst[:, 0:csz], idsb[:, 0:128],
                            sb[f"w_{st}"][:, c0 : c0 + csz],
                            start=True, stop=False,
                        ),
                        wait=mm1_wait,
                    )
                    pe(
                        f"mmB{t}_{st}_c{ci}",
                        lambda e, t=t, st=st, c0=c0, csz=csz, pst=pst: e.matmul(
                            pst[:, 0:csz], idsb[:, 128:256],
                            S(t, st)[:, c0 : c0 + csz],
                            start=False, stop=True,
                        ),
                    )
                    act(
                        f"t2c{t}_{st}_c{ci}",
                        lambda e, st=st, c0=c0, csz=csz, pst=pst: e.activation(
                            sb[f"t1_{st}"][:, c0 : c0 + csz], pst[:, 0:csz],
                            Act.Tanh, scale=0.5,
                        ),
                        wait=mark[f"mmB{t}_{st}_c{ci}"],
                    )
                    act(
                        f"cp{t}_{st}_c{ci}",
                        lambda e, st=st, c0=c0, csz=csz, pst=pst: e.activation(
                            sb[f"m_{st}"][:, c0 : c0 + csz], pst[:, 0:csz],
                            Act.Copy, bias=0.0, scale=1.0,
                        ),
                    )
                    last_cp[(st, bank)] = mark[f"cp{t}_{st}_c{ci}"]
                mark[f"t2_{t}_{st}"] = mark[f"t2c{t}_{st}_c{len(CHUNKS) - 1}"]
        else:
            for st in range(NSTREAM):
                # ACT: t2 = tanh(0.5 * m) (into t1 buffer)
                act(
                    f"t2_{t}_{st}",
                    lambda e, st=st: e.activation(
                        sb[f"t1_{st}"][:], sb[f"m_{st}"][:], Act.Tanh, scale=0.5
                    ),
                    wait=mark[f"m{t}_{st}"],
                )
        for st in range(NSTREAM):
            # w2 = (t2 + 1) * s ; z = (w2 * 0.5) + vt
            dve(
                f"w2{t}_{st}",
                lambda e, t=t, st=st: e.scalar_tensor_tensor(
                    sb[f"w_{st}"][:], sb[f"t1_{st}"][:], 1.0, S(t, st)[:],
                    Alu.add, Alu.mult,
                ),
                wait=mark[f"t2_{t}_{st}"],
            )
            dve(
                f"z{t}_{st}",
                lambda e, st=st: e.scalar_tensor_tensor(
                    sb[f"z_{st}"][:], sb[f"w_{st}"][:], 0.5, sb[f"vt_{st}"][:],
                    Alu.mult, Alu.add,
                ),
            )
            store(t, st)
        if t == 1:
            for st in range(NSTREAM):
                # x3 reuses X1; last X1 consumer was v1
                load(3, st, wait=mark[f"v1_{st}"])

    # ===================== t = 3 =========================================
    for st in range(NSTREAM):
        dve(
            f"v3_{st}",
            lambda e, st=st: e.scalar_tensor_tensor(
                sb[f"v_{st}"][:], sb[f"z_{st}"][:], -0.5, X(3, st)[:],
                Alu.mult, Alu.add,
            ),
            wait=mark[f"ld3_{st}"],
        )
    # split the final spike tiles so their stores drain during the epilogue
    half = FD // 2
    for st in range(NSTREAM):
        for h, (h0, hsz) in enumerate(((0, half), (half, FD - half))):
            dve(
                f"s3_{st}_h{h}",
                lambda e, st=st, h0=h0, hsz=hsz: e.tensor_scalar(
                    S(3, st)[:, h0 : h0 + hsz], sb[f"v_{st}"][:, h0 : h0 + hsz],
                    1.0, None, Alu.is_ge,
                ),
                wait=mark[f"st1_{st}"] if h == 0 else None,
            )
            emit(
                "scalar",
                lambda e, st=st, h0=h0, hsz=hsz: e.dma_start(
                    out=s_ext[3][:, FD * st + h0 : FD * st + h0 + hsz],
                    in_=S(3, st)[:, h0 : h0 + hsz],
                ),
                wait=mark[f"s3_{st}_h{h}"],
                inc=("so", 16),
                label=f"st3_{st}_h{h}",
            )

    # ---------------------------------------------------------------------
    final_so = counts["so"]
    with (
        nc.Block() as block,
        nc.semaphore("ld") as ld_sem,
        nc.semaphore("so") as so_sem,
        nc.semaphore("vec") as vec_sem,
        nc.semaphore("act") as act_sem,
        nc.semaphore("gps") as gps_sem,
        nc.semaphore("pe") as pe_sem,
    ):
        sems = {"ld": ld_sem, "so": so_sem, "vec": vec_sem, "act": act_sem,
                "gps": gps_sem, "pe": pe_sem}

        def run_plan(engine_handle, plan, final_wait=None):
            for fn, wait, inc in plan:
                ins = fn(engine_handle)
                if ins is None:
                    assert wait is not None and inc is None
                    engine_handle.wait_ge(sems[wait[0]], wait[1])
                    continue
                if wait is not None:
                    ins._wait_ge(sems[wait[0]], wait[1])
                if inc is not None:
                    ins.then_inc(sems[inc[0]], inc[1])
            if final_wait is not None:
                engine_handle.wait_ge(sems[final_wait[0]], final_wait[1])

        @block.sync
        def _(e):
            run_plan(e, plans["sync"])

        @block.tensor
        def _(e):
            run_plan(e, plans["tensor"])

        @block.gpsimd
        def _(e):
            run_plan(e, plans["gpsimd"])

        @block.vector
        def _(e):
            run_plan(e, plans["vector"])

        @block.scalar
        def _(e):
            run_plan(e, plans["scalar"], final_wait=("so", final_so))

    ctx.close()
    return nc


def _get_program():
    if "nc" not in _CACHE:
        _ensure_axon_hooks()
        _CACHE["nc"] = build_bass()
    return _CACHE["nc"]


def shard_inputs(x_seq):
    """x_seq [(t*b), n, c] -> per-core [STEP, P, FDFULL] contiguous blocks."""
    xt = np.ascontiguousarray(x_seq).reshape(STEP, B, N * C)
    if USE_PE:
        ids = np.zeros((P, 256), dtype=np.float32)
        ids[:, 0:128] = 0.5 * np.eye(P, dtype=np.float32)
        ids[:, 128:256] = np.eye(P, dtype=np.float32)
    maps = []
    for k in range(NCORES):
        blk = xt[:, k * BPC : (k + 1) * BPC, :].reshape(STEP, P, FDFULL)
        m = {"x": np.ascontiguousarray(blk)}
        if USE_PE:
            m["ids"] = ids.copy()
        maps.append(m)
    return maps


def unshard_outputs(results):
    """Per-core [STEP, P, FDFULL] spike blocks -> [(t*b), n, c]."""
    out = np.empty((STEP, B, N * C), dtype=np.float32)
    for k in range(NCORES):
        blk = results[k]["s"].reshape(STEP, BPC, N * C)
        out[:, k * BPC : (k + 1) * BPC, :] = blk
    return out.reshape(STEP * B, N, C)


def kernel(x_seq, step, _trace=False):
    assert int(step) == STEP
    assert x_seq.shape == (STEP * B, N, C)
    x_seq = np.asarray(x_seq, dtype=np.float32)

    from concourse.bass_utils import run_bass_kernel_spmd

    nc = _get_program()
    in_maps = shard_inputs(x_seq)
    res = run_bass_kernel_spmd(nc, in_maps, list(range(NCORES)), trace=_trace)
    out = unshard_outputs(res.results)
    if _trace:
        return out, res
    return out


# revision 28
# speedup vs baseline: 1.1961x; 1.0055x over previous
"""ComplementaryLIFNeuron on 8 Trainium2 NeuronCores (Bass, raw engine blocks).

Reference recurrence (per time step t, elementwise over [b, n, c]):
    v = v * 0.5 + x
    p = sigmoid(v / 2)          # 0.5 + 0.5*tanh(v/4)
    m = m * p
    s = (v >= 1)
    m = m + s
    q = sigmoid(m)              # 0.5 + 0.5*tanh(m/2)
    v = (v - s) - s * q
Output is s for each step, shape [(t*b), n, c].

Sharding: data-parallel over batch b=32 -> 4 rows per core; each (t, core)
block is a contiguous [4, 196*768] = [128, 4704] fp32 chunk, split into
2 interleaved column streams of 2352.

Exactness strategy (bit-identical to XLA fp32 except the tanh tables):
    m*p == 0.5 * ((tanh+1)*m)   (pow2 scaling commutes with RNE)
    track z = -((v-s) - s*q)    (negation symmetry of RNE), so
    s - v, + s*q and the next charge v' = (-0.5)*z + x all match exactly.
    t=0: v0 = x, s0 = (x>=1), m1 = s0, q0 = s0 * sigma(1) with sigma(1)
    hardcoded to XLA's fp32 bit pattern. t=3: only v3, s3 computed.

Engine split (per NeuronCore):
    DVE    : all elementwise chain ops (scalar_tensor_tensor / tensor_scalar);
             spikes are written as uint8 tiles (host converts to fp32)
    ACT    : the two tanh evaluations per step
    SCALAR : spike store DMA issue (HWDGE)
    SYNC   : input load DMA issue (HWDGE)
GPSIMD compute and TensorE offload were measured and rejected: GPSIMD
tensor ops run 2-19x slower than DVE and an exclusive SBUF-port lock
stalls concurrent DVE 2-input ops; the PE m-update (bit-exact via fp32r
identity matmuls) adds more PSUM-pipeline latency than it removes.
"""

import sys
import types
import numpy as np

STEP = 4
B = 32
N = 196
C = 768
NCORES = 8
BPC = B // NCORES            # batch rows per core = 4
PELEM = BPC * N * C          # elements per (t, core) block = 602112
P = 128                      # SBUF partitions
FDFULL = PELEM // P          # 4704 free-dim columns per (t, core)
NSTREAM = 2                  # independent column streams (interleaved)
FD = FDFULL // NSTREAM       # 2352 columns per stream tile

SIGMA1 = float(np.uint32(0x3F3B26A8).view(np.float32))  # XLA fp32 sigmoid(1.0)

USE_PE = False        # TensorEngine m-update: bit-exact but net slower
CHUNK = 512           # PSUM bank = 512 fp32 columns
CHUNKS = [(c, min(CHUNK, FD - c)) for c in range(0, FD, CHUNK)]

_CACHE = {}


def _ensure_axon_hooks():
    """bass_utils' trace path imports antenv.axon_hooks, absent in this image.

    Recreate the module and register the ctypes NTFF hook that
    trn_agent_boot would have installed if the module existed.
    """
    import antenv

    if "antenv.axon_hooks" not in sys.modules:
        m = types.ModuleType("antenv.axon_hooks")
        hook = [None]
        m.set_axon_ntff_profile_hook = lambda h: hook.__setitem__(0, h)
        m.get_axon_ntff_profile_hook = lambda: hook[0]
        sys.modules["antenv.axon_hooks"] = m
        antenv.axon_hooks = m
        try:
            from trn_agent_boot.trn_boot import _ntff_profile_via_ctypes

            h = _ntff_profile_via_ctypes("/opt/axon/libaxon_pjrt.so")
            if h is not None:
                m.set_axon_ntff_profile_hook(h)
        except Exception:
            pass


def build_bass():
    """Build the per-core SPMD Bass program."""
    from concourse import bass
    import concourse.mybir as mybir

    fp32 = mybir.dt.float32
    Alu = mybir.AluOpType
    Act = mybir.ActivationFunctionType

    nc = bass.Bass()
    x_ext = nc.declare_dram_parameter("x", [STEP, P, FDFULL], fp32, isOutput=False)
    s_ext = nc.declare_dram_parameter("s", [STEP, P, FDFULL], mybir.dt.uint8, isOutput=True)
    if USE_PE:
        # [0.5*I | I] stationary matrices for the PE m-update
        ids_ext = nc.declare_dram_parameter("ids", [P, 256], fp32, isOutput=False)

    import contextlib

    ctx = contextlib.ExitStack()
    sb = {}

    def tile(name):
        ap = ctx.enter_context(nc.sbuf_tensor(name, [P, FD], fp32))
        sb[name] = ap
        return ap

    for st in range(NSTREAM):
        for nm in ("X0", "X1", "z", "v", "t1", "w", "vt", "m"):
            tile(f"{nm}_{st}")
        for nm in ("S0", "S1"):
            ap = ctx.enter_context(
                nc.sbuf_tensor(f"{nm}_{st}", [P, FD], mybir.dt.uint8)
            )
            sb[f"{nm}_{st}"] = ap
    ps = {}
    if USE_PE:
        idsb = ctx.enter_context(nc.sbuf_tensor("idsb", [P, 256], fp32))
        for st in range(NSTREAM):
            for b in range(2):
                ps[(st, b)] = ctx.enter_context(
                    nc.psum_tensor(f"ps{st}_{b}", [P, CHUNK], fp32)
                )

    # Per-engine plans: (emit_fn, wait, inc); wait/inc = (sem_name, value)|None.
    plans = {"sync": [], "gpsimd": [], "vector": [], "scalar": [], "tensor": []}
    counts = {"ld": 0, "so": 0, "vec": 0, "act": 0, "gps": 0, "pe": 0}
    mark = {}     # label -> (sem, value) after that op
    last_cp = {}  # (stream, psum bank) -> mark of last ACT copy consuming it

    def emit(engine, fn, wait=None, inc=None, label=None):
        plans[engine].append((fn, wait, inc))
        if inc is not None:
            counts[inc[0]] += inc[1]
            if label is not None:
                mark[label] = (inc[0], counts[inc[0]])

    def xsrc(t, st):
        return x_ext[t][:, FD * st : FD * (st + 1)]

    def sdst(t, st):
        return s_ext[t][:, FD * st : FD * (st + 1)]

    X = lambda t, st: sb[f"X{t % 2}_{st}"]
    S = lambda t, st: sb[f"S{t % 2}_{st}"]

    def dve(label, fn, wait=None):
        emit("vector", fn, wait=wait, inc=("vec", 1), label=label)

    def gps(label, fn, wait=None):
        emit("gpsimd", fn, wait=wait, inc=("gps", 1), label=label)

    def act(label, fn, wait=None):
        emit("scalar", fn, wait=wait, inc=("act", 1), label=label)

    def pe(label, fn, wait=None):
        emit("tensor", fn, wait=wait, inc=("pe", 1), label=label)

    def load(t, st, wait=None):
        emit(
            "sync",
            lambda e, t=t, st=st: e.dma_start(out=X(t, st)[:], in_=xsrc(t, st)),
            wait=wait,
            inc=("ld", 16),
            label=f"ld{t}_{st}",
        )

    def store(t, st):
        emit(
            "scalar",
            lambda e, t=t, st=st: e.dma_start(out=sdst(t, st), in_=S(t, st)[:]),
            wait=mark[f"s{t}_{st}"],
            inc=("so", 16),
            label=f"st{t}_{st}",
        )

    def wait_only(engine, wait):
        emit(engine, lambda e: None, wait=wait)

    # --- prefetch x0, x1 for both streams -------------------------------
    QS = 588  # early-start slice of stream 0's first tile
    emit(
        "sync",
        lambda e: e.dma_start(out=X(0, 0)[:, 0:QS], in_=xsrc(0, 0)[:, 0:QS]),
        inc=("ld", 16),
        label="ld0_0a",
    )
    emit(
        "sync",
        lambda e: e.dma_start(out=X(0, 0)[:, QS:FD], in_=xsrc(0, 0)[:, QS:FD]),
        inc=("ld", 16),
        label="ld0_0",
    )
    load(0, 1)
    for st in range(NSTREAM):
        load(1, st)
    if USE_PE:
        emit(
            "sync",
            lambda e: e.dma_start(out=idsb[:], in_=ids_ext[:]),
            inc=("ld", 16),
            label="ldids",
        )
        # gate the whole PE plan on the identity matrices being resident
        wait_only("tensor", mark["ldids"])

    # ===================== t = 0 =========================================
    # DVE: s0 = (x>=1), vt0 = (x>=1) - x, z0 = vt0 + u0;  ACT: u0 = s0*sigma1
    # m1 aliases S0.
    for st in range(NSTREAM):
        if st == 0:
            dve(
                "s0_0a",
                lambda e: e.tensor_scalar(
                    S(0, 0)[:, 0:QS], X(0, 0)[:, 0:QS], 1.0, None, Alu.is_ge
                ),
                wait=mark["ld0_0a"],
            )
            dve(
                "vt0_0a",
                lambda e: e.scalar_tensor_tensor(
                    sb["vt_0"][:, 0:QS], X(0, 0)[:, 0:QS], 1.0,
                    X(0, 0)[:, 0:QS], Alu.is_ge, Alu.subtract,
                ),
            )
            dve(
                "s0_0",
                lambda e: e.tensor_scalar(
                    S(0, 0)[:, QS:FD], X(0, 0)[:, QS:FD], 1.0, None, Alu.is_ge
                ),
                wait=mark["ld0_0"],
            )
            dve(
                "vt0_0",
                lambda e: e.scalar_tensor_tensor(
                    sb["vt_0"][:, QS:FD], X(0, 0)[:, QS:FD], 1.0,
                    X(0, 0)[:, QS:FD], Alu.is_ge, Alu.subtract,
                ),
            )
        else:
            dve(
                f"s0_{st}",
                lambda e, st=st: e.tensor_scalar(
                    S(0, st)[:], X(0, st)[:], 1.0, None, Alu.is_ge
                ),
                wait=mark[f"ld0_{st}"],
            )
            dve(
                f"vt0_{st}",
                lambda e, st=st: e.scalar_tensor_tensor(
                    sb[f"vt_{st}"][:], X(0, st)[:], 1.0, X(0, st)[:],
                    Alu.is_ge, Alu.subtract,
                ),
            )
    for st in range(NSTREAM):
        dve(
            f"z0_{st}",
            lambda e, st=st: e.scalar_tensor_tensor(
                sb[f"z_{st}"][:], S(0, st)[:], SIGMA1, sb[f"vt_{st}"][:],
                Alu.mult, Alu.add,
            ),
        )
        store(0, st)

    # prefetch x2 (reuses X0; last X0 consumer at t=0 is vt0)
    for st in range(NSTREAM):
        load(2, st, wait=mark[f"vt0_{st}"])

    # ===================== t = 1, 2 ======================================
    for t in (1, 2):
        for st in range(NSTREAM):
            # v = (z * -0.5) + x
            dve(
                f"v{t}_{st}",
                lambda e, t=t, st=st: e.scalar_tensor_tensor(
                    sb[f"v_{st}"][:], sb[f"z_{st}"][:], -0.5, X(t, st)[:],
                    Alu.mult, Alu.add,
                ),
                wait=mark[f"ld{t}_{st}"],
            )
        for st in range(NSTREAM):
            # ACT: t1 = tanh(0.25 * v)
            act(
                f"t1_{t}_{st}",
                lambda e, st=st: e.activation(
                    sb[f"t1_{st}"][:], sb[f"v_{st}"][:], Act.Tanh, scale=0.25
                ),
                wait=mark[f"v{t}_{st}"],
            )
            # s = (v>=1) into S buffer; vt = (v>=1) - v
            swait = mark[f"st{t-2}_{st}"] if t >= 2 else None
            dve(
                f"s{t}_{st}",
                lambda e, t=t, st=st: e.tensor_scalar(
                    S(t, st)[:], sb[f"v_{st}"][:], 1.0, None, Alu.is_ge
                ),
                wait=swait,
            )
            dve(
                f"vt{t}_{st}",
                lambda e, st=st: e.scalar_tensor_tensor(
                    sb[f"vt_{st}"][:], sb[f"v_{st}"][:], 1.0, sb[f"v_{st}"][:],
                    Alu.is_ge, Alu.subtract,
                ),
            )
        for st in range(NSTREAM):
            mprev = S(0, st) if t == 1 else sb[f"m_{st}"]
            # w = (t1 + 1) * m_prev
            dve(
                f"w{t}_{st}",
                lambda e, st=st, mprev=mprev: e.scalar_tensor_tensor(
                    sb[f"w_{st}"][:], sb[f"t1_{st}"][:], 1.0, mprev[:],
                    Alu.add, Alu.mult,
                ),
                wait=mark[f"t1_{t}_{st}"],
            )
            if not USE_PE:
                # m = (w * 0.5) + s
                dve(
                    f"m{t}_{st}",
                    lambda e, t=t, st=st: e.scalar_tensor_tensor(
                        sb[f"m_{st}"][:], sb[f"w_{st}"][:], 0.5, S(t, st)[:],
                        Alu.mult, Alu.add,
                    ),
                )
        if USE_PE:
            # PE: m = 0.5*w + s per 512-col PSUM chunk; ACT: t2 = tanh(0.5*m)
            # from PSUM + bit-preserving copy of m back to SBUF.
            for st in range(NSTREAM):
                for ci, (c0, csz) in enumerate(CHUNKS):
                    bank = ci % 2
                    pst = ps[(st, bank)]
                    if ci == 0:
                        # vec(w) also transitively implies this bank's previous
                        # ACT copy finished (w waits t1 which follows it).
                        mm1_wait = mark[f"w{t}_{st}"]
                    else:
                        mm1_wait = last_cp.get((st, bank))
                    pe(
                        f"mmA{t}_{st}_c{ci}",
                        lambda e, st=st, c0=c0, csz=csz, pst=pst: e.matmul(
                            pst[:, 0:csz], idsb[:, 0:128],
                            sb[f"w_{st}"][:, c0 : c0 + csz],
                            start=True, stop=False,
                        ),
                        wait=mm1_wait,
                    )
                    pe(
                        f"mmB{t}_{st}_c{ci}",
                        lambda e, t=t, st=st, c0=c0, csz=csz, pst=pst: e.matmul(
                            pst[:, 0:csz], idsb[:, 128:256],
                            S(t, st)[:, c0 : c0 + csz],
                            start=False, stop=True,
                        ),
                    )
                    act(
                        f"t2c{t}_{st}_c{ci}",
                        lambda e, st=st, c0=c0, csz=csz, pst=pst: e.activation(
                            sb[f"t1_{st}"][:, c0 : c0 + csz], pst[:, 0:csz],
                            Act.Tanh, scale=0.5,
                        ),
                        wait=mark[f"mmB{t}_{st}_c{ci}"],
                    )
                    act(
                        f"cp{t}_{st}_c{ci}",
                        lambda e, st=st, c0=c0, csz=csz, pst=pst: e.activation(
                            sb[f"m_{st}"][:, c0 : c0 + csz], pst[:, 0:csz],
                            Act.Copy, bias=0.0, scale=1.0,
                        ),
                    )
                    last_cp[(st, bank)] = mark[f"cp{t}_{st}_c{ci}"]
                mark[f"t2_{t}_{st}"] = mark[f"t2c{t}_{st}_c{len(CHUNKS) - 1}"]
        else:
            for st in range(NSTREAM):
                # ACT: t2 = tanh(0.5 * m) (into t1 buffer)
                act(
                    f"t2_{t}_{st}",
                    lambda e, st=st: e.activation(
                        sb[f"t1_{st}"][:], sb[f"m_{st}"][:], Act.Tanh, scale=0.5
                    ),
                    wait=mark[f"m{t}_{st}"],
                )
        for st in range(NSTREAM):
            # w2 = (t2 + 1) * s ; z = (w2 * 0.5) + vt
            dve(
                f"w2{t}_{st}",
                lambda e, t=t, st=st: e.scalar_tensor_tensor(
                    sb[f"w_{st}"][:], sb[f"t1_{st}"][:], 1.0, S(t, st)[:],
                    Alu.add, Alu.mult,
                ),
                wait=mark[f"t2_{t}_{st}"],
            )
            dve(
                f"z{t}_{st}",
                lambda e, st=st: e.scalar_tensor_tensor(
                    sb[f"z_{st}"][:], sb[f"w_{st}"][:], 0.5, sb[f"vt_{st}"][:],
                    Alu.mult, Alu.add,
                ),
            )
            store(t, st)
        if t == 1:
            for st in range(NSTREAM):
                # x3 reuses X1; last X1 consumer was v1
                load(3, st, wait=mark[f"v1_{st}"])

    # ===================== t = 3 =========================================
    for st in range(NSTREAM):
        dve(
            f"v3_{st}",
            lambda e, st=st: e.scalar_tensor_tensor(
                sb[f"v_{st}"][:], sb[f"z_{st}"][:], -0.5, X(3, st)[:],
                Alu.mult, Alu.add,
            ),
            wait=mark[f"ld3_{st}"],
        )
    # split the final spike tiles so their stores drain during the epilogue
    half = FD // 2
    for st in range(NSTREAM):
        for h, (h0, hsz) in enumerate(((0, half), (half, FD - half))):
            dve(
                f"s3_{st}_h{h}",
                lambda e, st=st, h0=h0, hsz=hsz: e.tensor_scalar(
                    S(3, st)[:, h0 : h0 + hsz], sb[f"v_{st}"][:, h0 : h0 + hsz],
                    1.0, None, Alu.is_ge,
                ),
                wait=mark[f"st1_{st}"] if h == 0 else None,
            )
            emit(
                "scalar",
                lambda e, st=st, h0=h0, hsz=hsz: e.dma_start(
                    out=s_ext[3][:, FD * st + h0 : FD * st + h0 + hsz],
                    in_=S(3, st)[:, h0 : h0 + hsz],
                ),
                wait=mark[f"s3_{st}_h{h}"],
                inc=("so", 16),
                label=f"st3_{st}_h{h}",
            )

    # ---------------------------------------------------------------------
    final_so = counts["so"]
    with (
        nc.Block() as block,
        nc.semaphore("ld") as ld_sem,
        nc.semaphore("so") as so_sem,
        nc.semaphore("vec") as vec_sem,
        nc.semaphore("act") as act_sem,
        nc.semaphore("gps") as gps_sem,
        nc.semaphore("pe") as pe_sem,
    ):
        sems = {"ld": ld_sem, "so": so_sem, "vec": vec_sem, "act": act_sem,
                "gps": gps_sem, "pe": pe_sem}

        def run_plan(engine_handle, plan, final_wait=None):
            for fn, wait, inc in plan:
                ins = fn(engine_handle)
                if ins is None:
                    assert wait is not None and inc is None
                    engine_handle.wait_ge(sems[wait[0]], wait[1])
                    continue
                if wait is not None:
                    ins._wait_ge(sems[wait[0]], wait[1])
                if inc is not None:
                    ins.then_inc(sems[inc[0]], inc[1])
            if final_wait is not None:
                engine_handle.wait_ge(sems[final_wait[0]], final_wait[1])

        @block.sync
        def _(e):
            run_plan(e, plans["sync"])

        @block.tensor
        def _(e):
            run_plan(e, plans["tensor"])

        @block.gpsimd
        def _(e):
            run_plan(e, plans["gpsimd"])

        @block.vector
        def _(e):
            run_plan(e, plans["vector"])

        @block.scalar
        def _(e):
            run_plan(e, plans["scalar"], final_wait=("so", final_so))

    ctx.close()
    return nc


def _get_program():
    if "nc" not in _CACHE:
        _ensure_axon_hooks()
        _CACHE["nc"] = build_bass()
    return _CACHE["nc"]


def shard_inputs(x_seq):
    """x_seq [(t*b), n, c] -> per-core [STEP, P, FDFULL] contiguous blocks."""
    xt = np.ascontiguousarray(x_seq).reshape(STEP, B, N * C)
    if USE_PE:
        ids = np.zeros((P, 256), dtype=np.float32)
        ids[:, 0:128] = 0.5 * np.eye(P, dtype=np.float32)
        ids[:, 128:256] = np.eye(P, dtype=np.float32)
    maps = []
    for k in range(NCORES):
        blk = xt[:, k * BPC : (k + 1) * BPC, :].reshape(STEP, P, FDFULL)
        m = {"x": np.ascontiguousarray(blk)}
        if USE_PE:
            m["ids"] = ids.copy()
        maps.append(m)
    return maps


def unshard_outputs(results):
    """Per-core [STEP, P, FDFULL] spike blocks -> [(t*b), n, c]."""
    out = np.empty((STEP, B, N * C), dtype=np.float32)
    for k in range(NCORES):
        blk = results[k]["s"].reshape(STEP, BPC, N * C)
        out[:, k * BPC : (k + 1) * BPC, :] = blk
    return out.reshape(STEP * B, N, C)


def kernel(x_seq, step, _trace=False):
    assert int(step) == STEP
    assert x_seq.shape == (STEP * B, N, C)
    x_seq = np.asarray(x_seq, dtype=np.float32)

    from concourse.bass_utils import run_bass_kernel_spmd

    nc = _get_program()
    in_maps = shard_inputs(x_seq)
    res = run_bass_kernel_spmd(nc, in_maps, list(range(NCORES)), trace=_trace)
    out = unshard_outputs(res.results)
    if _trace:
        return out, res
    return out
